# revision 1
# baseline (speedup 1.0000x reference)
"""Trainium2 Bass kernel for local cost-volume correlation (FlowNet-style).

Problem: in1, in2 [B=8, C=256, H=96, W=128] fp32; out [B, 81, H, W] where
out[b, dy*9+dx, h, w] = mean_c in1[b,c,h,w] * in2[b,c,h+dy-4,w+dx-4] (zero pad).

Sharding: data-parallel over batch, one image per NeuronCore (8 cores).

Per-core algorithm (two output rows h per iteration):
  - PE computes Gram bands with 4 column-group (tile_position) matmuls per
    row, M=32 each: psum[w2, slot, u_local] = sum_c in1[c,h,w2] *
    in2pad[c,row(slot),u] where group j streams the u-window [32j, 32j+40)
    -- a free mod-32 shear.  in1 row is the stationary operand (fp16), the
    9 zero-padded in2 rows (rolling slot buffer) are the moving operand.
  - The needed values sit on diagonals u_local = (w2%32) + dxi.  Extraction
    refines the shear in stages: GPSIMD indirect_copy #1 (per-16-partition
    block offsets) -> mod-16; DVE stream_shuffle regroups partitions so each
    16-block holds a single (w2%16)//8 parity; indirect_copy #2 (per-block
    offsets, with the dy slot rotation folded into per-h index tables) ->
    mod-8.  One DVE masked multiply (mask[p, jj] = 1/C iff jj == p%8) +
    segmented reduce then extracts the 81 values per pixel exactly.
  - PE transpose whose "identity" is the inverse partition permutation
    assembles [81, h, w] output directly in dy-major channel order.
"""

import threading

import numpy as np

B, C, H, W = 8, 256, 96, 128
ND = 9            # displacement range per axis
NCH = ND * ND     # 81 output channels
CK = 2            # C // 128 contraction chunks
P = 128
NSLOT = 10        # rolling in2 row slots (2-row batching needs h-4..h+5)
SROW = 140        # padded in2 row width (>= 32*3 + 40)
UW = 40           # per-column-group u window
SW = NSLOT * UW   # 400, gram band row per h (after mod-32 shear)
PSROW = 512       # psum row pitch (bank-sized) for the 2-row tile
JW = 24           # gather1 width per slot (16 block + 8 disp)
G1ROW = ND * JW   # 216, gather1 output per row
NIDX = 2 * G1ROW  # 432 (two rows), multiple of 16
J2W = 16          # gather2 width per slot (8 block + 8 disp)
G2ROW = ND * J2W  # 144, gather2 output per row
NIDX2 = 2 * G2ROW  # 288
MW = 8            # mask window width after mod-8 shear

# stream_shuffle mask: group same (t//8) parity within each 32-quadrant
SHUF = list(range(0, 8)) + list(range(16, 24)) + list(range(8, 16)) + list(range(24, 32))

_cache = {}
_lock = threading.Lock()


def _wrap_idx(flat):
    """flat [8, n] per-core index lists -> wrapped [128, n//16] tensor."""
    n = flat.shape[1]
    out = np.zeros((P, n // 16), dtype=np.uint16)
    for q in range(8):
        for i in range(n):
            out[16 * q + (i % 16), i // 16] = flat[q, i]
    return out


def _host_tables():
    # gather1 (2 rows): slot selection + dy rotation folded in.  5 tables
    # indexed by hm = h % 10 (h even): row `row`, dy -> physical slot
    # (h + row + dy - 4) % 10; group q gathers
    # S[p, row*SW + slot_in*UW + 16*(q%2) + j], j in [0,24), dy-major output.
    tabs1 = []
    for hm in range(0, NSLOT, 2):
        flat1 = np.zeros((8, NIDX), dtype=np.uint16)
        for q in range(8):
            for row in range(2):
                for dy in range(ND):
                    slot_in = (hm + row + dy - 4) % NSLOT
                    for j in range(JW):
                        flat1[q, row * G1ROW + dy * JW + j] = (
                            row * SW + slot_in * UW + 16 * (q % 2) + j
                        )
        tabs1.append(_wrap_idx(flat1))
    gidx = np.stack(tabs1, axis=1).reshape(P, 5 * (NIDX // 16))

    # gather2 (after shuffle), static: group q gathers
    # qs[p, row*G1ROW + dy*JW + 8*(q%2) + j2]
    flat2 = np.zeros((8, NIDX2), dtype=np.uint16)
    for q in range(8):
        for row in range(2):
            for dy in range(ND):
                for j2 in range(J2W):
                    flat2[q, row * G2ROW + dy * J2W + j2] = (
                        row * G1ROW + dy * JW + 8 * (q % 2) + j2
                    )
    gidx2 = _wrap_idx(flat2)

    mask = np.zeros((P, MW), dtype=np.float16)
    for p in range(P):
        mask[p, p % 8] = 1.0 / C
    # inverse shuffle permutation matrix: perm[p_new, old(p_new)] = 1
    perm = np.zeros((P, P), dtype=np.float16)
    for s in range(4):
        for i in range(32):
            perm[32 * s + i, 32 * s + SHUF[i]] = 1.0
    return gidx, gidx2, mask, perm


def _build_nc():
    from contextlib import ExitStack

    import concourse.bass as bass
    import concourse.mybir as mybir
    import concourse.tile as tile
    from concourse import bacc

    f32 = mybir.dt.float32
    f16 = mybir.dt.float16
    u16 = mybir.dt.uint16

    nc = bacc.Bacc("TRN2", target_bir_lowering=False, debug=False)
    in1 = nc.declare_dram_parameter("in1", [C, H, W], f32, isOutput=False)
    in2 = nc.declare_dram_parameter("in2", [C, H, W], f32, isOutput=False)
    gidx = nc.declare_dram_parameter(
        "gidx", [P, 5 * (NIDX // 16)], u16, isOutput=False
    )
    gidx2 = nc.declare_dram_parameter("gidx2", [P, NIDX2 // 16], u16, isOutput=False)
    maskt = nc.declare_dram_parameter("maskt", [P, MW], f16, isOutput=False)
    permt = nc.declare_dram_parameter("permt", [P, P], f16, isOutput=False)
    out_t = nc.declare_dram_parameter("out", [NCH, H, W], f32, isOutput=True)

    in1r = in1[:].rearrange("(k p) h w -> p k h w", p=P)
    in2r = in2[:].rearrange("(k p) h w -> p k h w", p=P)

    with ExitStack() as ctx:
        tc = ctx.enter_context(tile.TileContext(nc))
        const = ctx.enter_context(tc.tile_pool(name="const", bufs=1))
        persist = ctx.enter_context(tc.tile_pool(name="persist", bufs=1))
        inp = ctx.enter_context(tc.tile_pool(name="inp", bufs=4))
        wrp = ctx.enter_context(tc.tile_pool(name="wrp", bufs=3))
        sp = ctx.enter_context(tc.tile_pool(name="sp", bufs=3))
        qp = ctx.enter_context(tc.tile_pool(name="qp", bufs=3))
        qsp = ctx.enter_context(tc.tile_pool(name="qsp", bufs=3))
        q8p = ctx.enter_context(tc.tile_pool(name="q8p", bufs=3))
        pp = ctx.enter_context(tc.tile_pool(name="pp", bufs=3))
        op = ctx.enter_context(tc.tile_pool(name="op", bufs=3))
        gram = ctx.enter_context(tc.tile_pool(name="gram", bufs=3, space="PSUM"))
        ptp = ctx.enter_context(tc.tile_pool(name="ptp", bufs=2, space="PSUM"))

        gidx1_tabs = []
        for it in range(5):
            g1 = const.tile([P, NIDX // 16], u16, name=f"g1_{it}")
            nc.sync.dma_start(
                out=g1[:],
                in_=gidx[:, it * (NIDX // 16) : (it + 1) * (NIDX // 16)],
            )
            gidx1_tabs.append(g1)
        gidx2_s = const.tile([P, NIDX2 // 16], u16)
        nc.sync.dma_start(out=gidx2_s[:], in_=gidx2[:])
        mask_s = const.tile([P, MW], f16)
        nc.sync.dma_start(out=mask_s[:], in_=maskt[:])
        perm_s = const.tile([P, P], f16)
        nc.sync.dma_start(out=perm_s[:], in_=permt[:])

        # rolling fp16 padded in2 rows: [p, k, slot, u]
        Rr = persist.tile([P, CK, NSLOT, SROW], f16)
        nc.vector.memset(Rr[:], 0.0)
        # output accumulator [channel, h, w]
        Tfull = persist.tile([NCH, H, W], f32)

        def load_cast_in2_pair(r):
            # loads rows r, r+1 (both must be < H)
            rin = inp.tile([P, CK, 2, W], f32, tag="rin")
            nc.sync.dma_start(out=rin[:], in_=in2r[:, :, r : r + 2, :])
            s = r % NSLOT
            if s + 1 <= NSLOT - 1:
                nc.gpsimd.tensor_copy(
                    out=Rr[:, :, s : s + 2, 4 : 4 + W], in_=rin[:]
                )
            else:  # slot wrap: 8 then 0
                nc.gpsimd.tensor_copy(
                    out=Rr[:, :, s, 4 : 4 + W], in_=rin[:, :, 0, :]
                )
                nc.gpsimd.tensor_copy(
                    out=Rr[:, :, 0, 4 : 4 + W], in_=rin[:, :, 1, :]
                )

        for r in range(0, 4, 2):
            load_cast_in2_pair(r)

        for h in range(0, H, 2):
            r = h + 4
            if r + 1 < H:
                load_cast_in2_pair(r)
            else:  # h = 92 or 94: rows r, r+1 >= 96 -> zero the slots
                nc.vector.memset(Rr[:, :, r % NSLOT, :], 0.0)
                nc.vector.memset(Rr[:, :, (r + 1) % NSLOT, :], 0.0)

            win = inp.tile([P, CK, 2, W], f32, tag="win")
            nc.sync.dma_start(out=win[:], in_=in1r[:, :, h : h + 2, :])
            wr = wrp.tile([P, CK, 2, W], f16)
            nc.scalar.copy(out=wr[:, 0], in_=win[:, 0])
            nc.vector.tensor_copy(out=wr[:, 1], in_=win[:, 1])

            # col-group matmuls: group j computes w2 in [32j, 32j+32) against
            # u in [32j, 32j+40) -> psum[32j:32j+32, row*PSROW + slot*40 + ...]
            ps = gram.tile([P, 2 * PSROW], f32)
            S = sp.tile([P, 2, SW], f16)
            for row in range(2):
                for k in range(CK):
                    for j in range(4):
                        nc.tensor.matmul(
                            out=ps[32 * j : 32 * j + 32, row * PSROW : row * PSROW + SW],
                            lhsT=wr[:, k, row, 32 * j : 32 * j + 32],
                            rhs=Rr[:, k, :, 32 * j : 32 * j + UW],
                            start=(k == 0),
                            stop=(k == CK - 1),
                            tile_position=(0, 32 * j),
                        )
                nc.scalar.copy(
                    out=S[:, row], in_=ps[:, row * PSROW : row * PSROW + SW]
                )

            q16 = qp.tile([P, NIDX], f16)
            nc.gpsimd.indirect_copy(
                out=q16[:], data=S[:].rearrange("p a b -> p (a b)"),
                idxs=gidx1_tabs[(h % NSLOT) // 2][:],
                i_know_ap_gather_is_preferred=True,
            )

            qs = qsp.tile([P, NIDX], f16)
            nc.vector.stream_shuffle(out=qs[:], in_=q16[:], mask=SHUF)

            q8 = q8p.tile([P, NIDX2], f16)
            nc.gpsimd.indirect_copy(
                out=q8[:], data=qs[:], idxs=gidx2_s[:],
                i_know_ap_gather_is_preferred=True,
            )

            # masked multiply:
            # prod[p, row, dy, dxi, jj] = q8[p, row*144 + dy*16 + dxi + jj] * mask[p, jj]
            prod = pp.tile([P, 2 * NCH * MW], f16)
            q8a = q8[:]
            # (row, dy) merged: row stride 144 = 9 * J2W, so one uniform dim
            in0 = bass.AP(
                tensor=q8a.tensor,
                offset=q8a.offset,
                ap=[q8a.ap[0], [J2W, 2 * ND], [1, ND], [1, MW]],
            )
            in1b = (
                mask_s[:]
                .unsqueeze(1)
                .unsqueeze(1)
                .to_broadcast([P, 2 * ND, ND, MW])
            )
            nc.vector.tensor_mul(
                prod[:].rearrange("p (a b c) -> p a b c", b=ND, c=MW),
                in0,
                in1b,
            )

            # pairwise add tree (tensor_tensor runs at 2x, tensor_reduce at 1x)
            pr4 = prod[:].rearrange("p (a c) -> p a c", c=MW)
            t1 = op.tile([P, 2 * NCH, 4], f16, name="t1", tag="t1")
            nc.vector.tensor_add(t1[:], pr4[:, :, 0:4], pr4[:, :, 4:8])
            t2 = op.tile([P, 2 * NCH, 2], f16, name="t2", tag="t2")
            nc.vector.tensor_add(t2[:], t1[:, :, 0:2], t1[:, :, 2:4])
            O = op.tile([P, 2 * NCH], f16)
            nc.vector.tensor_add(O[:], t2[:, :, 0], t2[:, :, 1])

            # transpose via PE; rhs = inverse shuffle permutation, so columns
            # land at the true pixel positions.
            for row in range(2):
                pt = ptp.tile([NCH, P], f16, name=f"pt{row}", tag="pt")
                nc.tensor.transpose(
                    out=pt[:], in_=O[:, row * NCH : (row + 1) * NCH],
                    identity=perm_s[:],
                )
                nc.scalar.copy(out=Tfull[:, h + row, :], in_=pt[:])

            if h % 8 == 6:
                nc.sync.dma_start(
                    out=out_t[:, h - 6 : h + 2, :], in_=Tfull[:, h - 6 : h + 2, :]
                )

    nc.finalize()
    return nc


def _get_nc():
    with _lock:
        if "nc" not in _cache:
            _cache["nc"] = _build_nc()
        return _cache["nc"]


def kernel(in1: np.ndarray, in2: np.ndarray) -> np.ndarray:
    from concourse.bass_utils import run_bass_kernel_spmd

    nc = _get_nc()
    gidx, gidx2, mask, perm = _host_tables()
    in1 = np.ascontiguousarray(in1, dtype=np.float32)
    in2 = np.ascontiguousarray(in2, dtype=np.float32)
    in_maps = [
        {
            "in1": in1[b],
            "in2": in2[b],
            "gidx": gidx,
            "gidx2": gidx2,
            "maskt": mask,
            "permt": perm,
        }
        for b in range(B)
    ]
    res = run_bass_kernel_spmd(nc, in_maps, core_ids=list(range(B)))
    out = np.stack([res.results[b]["out"] for b in range(B)], axis=0)
    return out



# revision 2
# speedup vs baseline: 1.3123x; 1.3123x over previous
"""Trainium2 Bass kernel for local cost-volume correlation (FlowNet-style).

Problem: in1, in2 [B=8, C=256, H=96, W=128] fp32; out [B, 81, H, W] where
out[b, dy*9+dx, h, w] = mean_c in1[b,c,h,w] * in2[b,c,h+dy-4,w+dx-4] (zero pad).

Sharding: data-parallel over batch, one image per NeuronCore (8 cores).

Per-core algorithm, two output rows per iteration, software-pipelined with a
stage skew so every engine streams its own stage back-to-back:

  stage    pair   engine  work
  load     i+1/2  DMA     in2 row pair / in1 row pair
  cast     i+1    DVE     f32->f16 into the 12-slot rolling in2 ring (Rr) /
                          in1 row pair (wr).  NSLOT=12 gives the ring a full
                          iteration of WAR slack vs the matmuls.
  gram     i      PE      4 col-group (tile_position) matmuls per (row,chunk),
                          M=32, N=12*40: psum[w, row, slot*40+u] with the free
                          mod-32 shear u = w%32 + dxi.
  Scopy    i      ACT     psum -> SBUF fp16 band S.
  g1       i-1    GPSIMD  ap_gather (u32 pairs): per-16-block 24-wide windows
                          -> mod-16 resolved, dy-major (slot rotation folded
                          into 6 per-h%12 index tables).
  sigma1   i-1    DVE     stream_shuffle (u32): regroup partitions so each
                          16-block holds one (w%16)//8 parity.
  g2       i-1    GPSIMD  ap_gather (u32): -> mod-8 resolved, 16-wide windows.
  sigma2   i-2    PE      matmul with a 1/C-scaled permutation as stationary:
                          full cross-32 re-sort to w%8-major partition order.
  qcopy    i-2    ACT     psum f32 -> SBUF.
  g3       i-2    GPSIMD  ap_gather (f32 singles): per-16-block offset is now
                          exact -> O[p, row*81 + dy*9 + dxi], already scaled.
  trans    i-3    PE      matmul(lhsT=O-slice, rhs=pixel-permutation) = output
                          row in [81, w] order, undoing both partition sorts.
  ocopy    i-3    ACT     psum -> rolling [81, 8, 128] buffer
  store    i-3    DMA     every 4th pair: 8 output rows -> HBM.
"""

import threading

import numpy as np

B, C, H, W = 8, 256, 96, 128
ND = 9             # displacement range per axis
NCH = ND * ND      # 81 output channels
CK = 2             # C // 128 contraction chunks
P = 128
NP = H // 2        # 48 row pairs
NSLOT = 12         # rolling in2 row slots (10 live + 2 slack for pipelining)
SROW = 140         # padded in2 row width
UW = 40            # per-column-group u window
SW = NSLOT * UW    # 480, gram band per row
PSROW = 512        # psum row pitch (bank aligned)
N1 = 216           # g1 real u32 outputs (2 rows * 9 dy * 12)
N1P = 224          # padded to %16
N2 = 144           # g2 u32 outputs (18 * 8)
N3 = 162           # g3 real f32 outputs (2 * 81)
N3P = 176          # padded to %16

# sigma1: out partition p holds old partition 32*(p//32) + SHUF[p%32]
SHUF = list(range(0, 8)) + list(range(16, 24)) + list(range(8, 16)) + list(range(24, 32))

_cache = {}
_lock = threading.Lock()


def _wrap_idx(flat, n):
    """flat [8, n] per-block index lists -> wrapped [128, n//16] int16."""
    out = np.zeros((P, n // 16), dtype=np.int16)
    for q in range(8):
        for i in range(n):
            out[16 * q + (i % 16), i // 16] = flat[q, i]
    return out


def _host_tables():
    # g1 (u32 pairs): 6 tables by hm = h % 12 (h even). dy-rotation folded in.
    tabs1 = []
    for hm in range(0, NSLOT, 2):
        flat = np.zeros((8, N1P), dtype=np.int16)
        for q in range(8):
            for i in range(N1):
                rd, jw = divmod(i, 12)
                row, dy = divmod(rd, ND)
                slot_in = (hm + row + dy - 4) % NSLOT
                flat[q, i] = row * 240 + slot_in * 20 + 8 * (q % 2) + jw
        tabs1.append(_wrap_idx(flat, N1P))
    gidx1 = np.concatenate(tabs1, axis=1)  # [128, 6*14]

    flat2 = np.zeros((8, N2), dtype=np.int16)
    for q in range(8):
        for i in range(N2):
            rd, jw = divmod(i, 8)
            flat2[q, i] = rd * 12 + 4 * (q % 2) + jw
    gidx2 = _wrap_idx(flat2, N2)  # [128, 9]

    flat3 = np.zeros((8, N3P), dtype=np.int16)
    for q in range(8):
        for i in range(N3):
            rd, dxi = divmod(i, 9)
            flat3[q, i] = rd * 16 + dxi + q
    gidx3 = _wrap_idx(flat3, N3P)  # [128, 11]

    # sigma2 permutation (1/C scaled): p1 = 32Q+16b+8hi+e -> p2 = 16e+4Q+2hi+b
    p2m = np.zeros((P, P), dtype=np.float16)
    for p1 in range(P):
        Q, r = divmod(p1, 32)
        b = r // 16
        hi, e = divmod(p1 % 16, 8)
        p2 = 16 * e + 4 * Q + 2 * hi + b
        p2m[p1, p2] = 1.0 / C
    # output pixel permutation: column w of the transposed output reads
    # partition p2 with w(p2) = 32*(m//4)+16*((m%4)//2)+8*(m%2)+p2//16
    pwm = np.zeros((P, P), dtype=np.float32)
    for p2 in range(P):
        m = p2 % 16
        w = 32 * (m // 4) + 16 * ((m % 4) // 2) + 8 * (m % 2) + p2 // 16
        pwm[p2, w] = 1.0
    return gidx1, gidx2, gidx3, p2m, pwm


def _input_map(in1_b, in2_b, tables):
    gidx1, gidx2, gidx3, p2m, pwm = tables
    return {
        "in1": in1_b,
        "in2": in2_b,
        "gidx1": gidx1,
        "gidx2": gidx2,
        "gidx3": gidx3,
        "p2m": p2m,
        "pwm": pwm,
    }


def _build_nc():
    from contextlib import ExitStack

    import concourse.bass as bass
    import concourse.mybir as mybir
    import concourse.tile as tile
    from concourse import bacc

    f32 = mybir.dt.float32
    f16 = mybir.dt.float16
    u32 = mybir.dt.uint32
    i16 = mybir.dt.int16

    nc = bacc.Bacc("TRN2", target_bir_lowering=False, debug=False)
    in1 = nc.declare_dram_parameter("in1", [C, H, W], f32, isOutput=False)
    in2 = nc.declare_dram_parameter("in2", [C, H, W], f32, isOutput=False)
    gidx1 = nc.declare_dram_parameter("gidx1", [P, 6 * (N1P // 16)], i16, isOutput=False)
    gidx2 = nc.declare_dram_parameter("gidx2", [P, N2 // 16], i16, isOutput=False)
    gidx3 = nc.declare_dram_parameter("gidx3", [P, N3P // 16], i16, isOutput=False)
    p2m = nc.declare_dram_parameter("p2m", [P, P], f16, isOutput=False)
    pwm = nc.declare_dram_parameter("pwm", [P, P], f32, isOutput=False)
    out_t = nc.declare_dram_parameter("out", [NCH, H, W], f32, isOutput=True)

    in1r = in1[:].rearrange("(k p) h w -> p k h w", p=P)
    in2r = in2[:].rearrange("(k p) h w -> p k h w", p=P)

    with ExitStack() as ctx:
        tc = ctx.enter_context(tile.TileContext(nc))
        const = ctx.enter_context(tc.tile_pool(name="const", bufs=1))
        persist = ctx.enter_context(tc.tile_pool(name="persist", bufs=1))
        rinp = ctx.enter_context(tc.tile_pool(name="rinp", bufs=3))
        winp = ctx.enter_context(tc.tile_pool(name="winp", bufs=3))
        wrp = ctx.enter_context(tc.tile_pool(name="wrp", bufs=3))
        sp = ctx.enter_context(tc.tile_pool(name="sp", bufs=3))
        q16p = ctx.enter_context(tc.tile_pool(name="q16p", bufs=3))
        qsp = ctx.enter_context(tc.tile_pool(name="qsp", bufs=3))
        q8p = ctx.enter_context(tc.tile_pool(name="q8p", bufs=3))
        q8sp = ctx.enter_context(tc.tile_pool(name="q8sp", bufs=3))
        op = ctx.enter_context(tc.tile_pool(name="op", bufs=3))
        tbp = ctx.enter_context(tc.tile_pool(name="tbp", bufs=2))
        gram = ctx.enter_context(tc.tile_pool(name="gram", bufs=2, space="PSUM"))
        psqp = ctx.enter_context(tc.tile_pool(name="psqp", bufs=2, space="PSUM"))
        ptp = ctx.enter_context(tc.tile_pool(name="ptp", bufs=2, space="PSUM"))

        g1tabs = []
        for it in range(6):
            g1t = const.tile([P, N1P // 16], i16, name=f"g1_{it}")
            nc.sync.dma_start(
                out=g1t[:],
                in_=gidx1[:, it * (N1P // 16) : (it + 1) * (N1P // 16)],
            )
            g1tabs.append(g1t)
        g2tab = const.tile([P, N2 // 16], i16)
        nc.sync.dma_start(out=g2tab[:], in_=gidx2[:])
        g3tab = const.tile([P, N3P // 16], i16)
        nc.sync.dma_start(out=g3tab[:], in_=gidx3[:])
        p2s = const.tile([P, P], f16)
        nc.sync.dma_start(out=p2s[:], in_=p2m[:])
        pws = const.tile([P, P], f32)
        nc.sync.dma_start(out=pws[:], in_=pwm[:])

        # rolling fp16 padded in2 rows: [p, k, slot, u]
        Rr = persist.tile([P, CK, NSLOT, SROW], f16)
        nc.vector.memset(Rr[:], 0.0)

        # ---- per-stage emitters -------------------------------------------
        rin_t = {}
        win_t = {}
        wr_t = {}
        s_t = {}
        q16_t = {}
        q8_t = {}
        q8s_t = {}
        o_t = {}
        ps_t = {}
        tb_t = {}

        def dma_rin(j):
            # in2 rows for pair j: 2j+4, 2j+5
            r = 2 * j + 4
            if r >= H:
                return
            rin = rinp.tile([P, CK, 2, W], f32, name="rin", tag="rin")
            nc.sync.dma_start(out=rin[:], in_=in2r[:, :, r : r + 2, :])
            rin_t[j] = rin

        def dma_win(j):
            win = winp.tile([P, CK, 2, W], f32, name="win", tag="win")
            nc.sync.dma_start(out=win[:], in_=in1r[:, :, 2 * j : 2 * j + 2, :])
            win_t[j] = win

        def cast_rin(j):
            r = 2 * j + 4
            s = r % NSLOT  # always even, never wraps for a pair
            if r >= H:
                nc.vector.memset(Rr[:, :, s : s + 2, :], 0.0)
                return
            nc.vector.tensor_copy(out=Rr[:, :, s : s + 2, 4 : 4 + W], in_=rin_t.pop(j)[:])

        def cast_win(j):
            wr = wrp.tile([P, CK, 2, W], f16, name="wr", tag="wr")
            nc.vector.tensor_copy(out=wr[:], in_=win_t.pop(j)[:])
            wr_t[j] = wr

        def gram_mm(j):
            ps = gram.tile([P, 2, PSROW], f32, name="ps", tag="ps")
            wr = wr_t.pop(j)
            for row in range(2):
                for k in range(CK):
                    for g in range(4):
                        nc.tensor.matmul(
                            out=ps[32 * g : 32 * g + 32, row, 0:SW],
                            lhsT=wr[:, k, row, 32 * g : 32 * g + 32],
                            rhs=Rr[:, k, :, 32 * g : 32 * g + UW],
                            start=(k == 0),
                            stop=(k == CK - 1),
                            tile_position=(0, 32 * g),
                        )
            ps_t[j] = ps

        def s_copy(j):
            ps = ps_t.pop(j)
            S = sp.tile([P, 2, SW], f16, name="S", tag="S")
            for row in range(2):
                nc.scalar.copy(out=S[:, row], in_=ps[:, row, 0:SW])
            s_t[j] = S

        def g1(j):
            S = s_t.pop(j)
            q16 = q16p.tile([P, N1P], u32, name="q16", tag="q16")
            nc.gpsimd.ap_gather(
                out_ap=q16[:],
                in_ap=S[:].rearrange("p a b -> p (a b)").bitcast(u32),
                idxs_ap=g1tabs[(2 * j % NSLOT) // 2][:],
                channels=P,
                num_elems=SW,  # 960 fp16 = 480 u32... SW*2/2
                num_idxs=N1P,
                d=1,
            )
            q16_t[j] = q16

        def sigma1(j):
            q16 = q16_t.pop(j)
            qs = qsp.tile([P, N1P], u32, name="qs", tag="qs")
            nc.vector.stream_shuffle(out=qs[:], in_=q16[:], mask=SHUF)
            q16_t[j] = qs

        def g2(j):
            qs = q16_t.pop(j)
            q8 = q8p.tile([P, N2], u32, name="q8", tag="q8")
            nc.gpsimd.ap_gather(
                out_ap=q8[:],
                in_ap=qs[:],
                idxs_ap=g2tab[:],
                channels=P,
                num_elems=N1P,
                num_idxs=N2,
                d=1,
            )
            q8_t[j] = q8

        def sigma2(j):
            q8 = q8_t.pop(j)
            psq = psqp.tile([P, 2 * N2], f32, name="psq", tag="psq")
            nc.tensor.matmul(
                out=psq[:],
                lhsT=p2s[:],
                rhs=q8[:].bitcast(f16),
                start=True,
                stop=True,
            )
            q8_t[j] = psq

        def q_copy(j):
            psq = q8_t.pop(j)
            q8s = q8sp.tile([P, 2 * N2], f32, name="q8s", tag="q8s")
            nc.scalar.copy(out=q8s[:], in_=psq[:])
            q8s_t[j] = q8s

        def g3(j):
            q8s = q8s_t.pop(j)
            O = op.tile([P, N3P], f32, name="O", tag="O")
            nc.gpsimd.ap_gather(
                out_ap=O[:],
                in_ap=q8s[:],
                idxs_ap=g3tab[:],
                channels=P,
                num_elems=2 * N2,
                num_idxs=N3P,
                d=1,
            )
            o_t[j] = O

        def trans(j):
            O = o_t.pop(j)
            if j % 4 == 0:
                tb_t[j // 4] = tbp.tile([NCH, 8, W], f32, name="tb", tag="tb")
            tb = tb_t[j // 4]
            for row in range(2):
                pt = ptp.tile([NCH, W], f32, name="pt", tag="pt")
                nc.tensor.matmul(
                    out=pt[:],
                    lhsT=O[:, row * NCH : (row + 1) * NCH],
                    rhs=pws[:],
                    start=True,
                    stop=True,
                )
                nc.scalar.copy(out=tb[:, (2 * j) % 8 + row, :], in_=pt[:])

        def store(j):
            tb = tb_t.pop(j // 4)
            nc.sync.dma_start(
                out=out_t[:, 2 * j - 6 : 2 * j + 2, :],
                in_=tb[:],
            )

        # ---- prologue -----------------------------------------------------
        # in2 rows 0..5 (pair 0 needs rows -4..5; negatives stay zero)
        for jj, r in ((None, 0), (None, 2)):
            rin = rinp.tile([P, CK, 2, W], f32, name="rin", tag="rin")
            nc.sync.dma_start(out=rin[:], in_=in2r[:, :, r : r + 2, :])
            nc.vector.tensor_copy(out=Rr[:, :, r : r + 2, 4 : 4 + W], in_=rin[:])
        dma_rin(0)   # rows 4,5
        dma_win(0)
        dma_win(1)
        cast_rin(0)
        cast_win(0)

        # ---- main skewed loop --------------------------------------------
        for i in range(NP + 3):
            if i + 1 < NP:
                dma_rin(i + 1)
            if i + 2 < NP:
                dma_win(i + 2)
            if i + 1 < NP:
                cast_rin(i + 1)
                cast_win(i + 1)
            if i < NP:
                gram_mm(i)
                s_copy(i)
            if 0 <= i - 1 < NP:
                g1(i - 1)
                sigma1(i - 1)
                g2(i - 1)
            if 0 <= i - 2 < NP:
                sigma2(i - 2)
                q_copy(i - 2)
                g3(i - 2)
            if 0 <= i - 3 < NP:
                trans(i - 3)
                if (i - 3) % 4 == 3:
                    store(i - 3)

    nc.finalize()
    return nc


def _get_nc():
    with _lock:
        if "nc" not in _cache:
            _cache["nc"] = _build_nc()
        return _cache["nc"]


def kernel(in1: np.ndarray, in2: np.ndarray) -> np.ndarray:
    from concourse.bass_utils import run_bass_kernel_spmd

    nc = _get_nc()
    tables = _host_tables()
    in1 = np.ascontiguousarray(in1, dtype=np.float32)
    in2 = np.ascontiguousarray(in2, dtype=np.float32)
    in_maps = [_input_map(in1[b], in2[b], tables) for b in range(B)]
    res = run_bass_kernel_spmd(nc, in_maps, core_ids=list(range(B)))
    out = np.stack([res.results[b]["out"] for b in range(B)], axis=0)
    return out
